# revision 54
# baseline (speedup 1.0000x reference)
"""GQA attention layer (dense_transformer) on 8 Trainium2 NeuronCores.

Tensor-parallel over heads: each core gets 4 q-heads + 1 kv-head (shard of
wq/wk/wv output dims and wo input dim), hidden_states replicated; partial
o_proj outputs are summed on the host (the all-reduce).

Structure (all matmuls bf16, fp32 PSUM; measured ~774us vs 1007us baseline):
  phase 1: per 128-token tile, qkv projections from host-PRETILED hsT
    and wq/wkv (contiguous multi-KB DMA lines per partition); 4-chunk
    weight-group DMAs + PE-warmup transposes so compute starts hot; fused
    RMSNorm+RoPE via scalar_tensor_tensor (rope add on GpSimd); the PE
    transposes of tile i are deferred into tile i+1's matmul stream so the
    in-order PE queue never waits on the DVE/GpSimd rope chain.
  phase 2: per batch, j-stripe (512 q cols) outer, head inner.
    scoresT = k @ qT per k-tile with causal width-trimming on diagonal
    tiles; exp on ACT (table preloaded in phase 1); triangular mask on DVE;
    PV split in two passes (s=0,1 then s=2,3) so only 2 PSUM banks hold PV
    accumulators; probs tiles persist in SBUF for pass B reuse.  o_proj
    2-matmul half-units are interleaved into the attention stream via a
    paced work queue so the PE stays busy during exp stalls.  o_proj
    results copied to bf16 on DVE (not ACT) and DMA'd out bf16; host sums
    partials in fp32 (the tensor-parallel all-reduce).
"""

import numpy as np
import ml_dtypes

H, KV, D, HID = 32, 8, 128, 4096
B, S = 2, 2048
T = B * S
NCORES = 8
HL = H // NCORES          # 4 q heads per core
QF = HL * D               # 512
EPS = 1e-6
THETA = 10000.0
SCALE = 1.0 / float(np.sqrt(D))

NT = T // 128             # 32 token tiles
NTB = S // 128            # 16 token tiles per batch
NC = HID // 128           # 32 contraction chunks

_NC_CACHE = {}


def _build(shared_tabs):
    import concourse.bacc as bacc
    import concourse.mybir as mybir
    import concourse.tile as tile
    from concourse.masks import make_identity

    fp32 = mybir.dt.float32
    bf16 = mybir.dt.bfloat16
    MUL = mybir.AluOpType.mult

    nc = bacc.Bacc("TRN2", target_bir_lowering=False)

    # host-pretiled: hsT[p, i, c, t] = hs[128*i + t, 128*c + p], so a token
    # tile's DMA is one contiguous 8KB run per partition
    hsT = nc.dram_tensor("hsT", [128, NT, NC, 128], bf16, kind="ExternalInput")
    # host-pretiled like hsT: wq[p, c, f] = wq_full[128*c + p, f]
    wq = nc.dram_tensor("wq", [128, NC, QF], bf16, kind="ExternalInput")
    wkv = nc.dram_tensor("wkv", [128, NC, 2 * D], bf16, kind="ExternalInput")
    wo = nc.dram_tensor("wo", [QF, HID], bf16, kind="ExternalInput")
    tab_names = ["cosq", "sinq"] if shared_tabs else ["cosq", "sinq", "cosk", "sink"]
    tabs_dram = {n: nc.dram_tensor(n, [S, D], bf16, kind="ExternalInput")
                 for n in tab_names}
    out = nc.dram_tensor("out", [T, HID], bf16, kind="ExternalOutput")

    with tile.TileContext(nc) as tc:
        with (
            tc.tile_pool(name="persist", bufs=1) as persist,
            tc.tile_pool(name="hst", bufs=2) as hstp,
            tc.tile_pool(name="work", bufs=3) as work,
            tc.tile_pool(name="prp", bufs=16) as prp,
            tc.tile_pool(name="stats", bufs=4) as stats,
            tc.tile_pool(name="ostage", bufs=3) as ostage,
            tc.tile_pool(name="psA", bufs=2, space="PSUM") as psA,
            tc.tile_pool(name="psB", bufs=2, space="PSUM") as psB,
            tc.tile_pool(name="psT", bufs=2, space="PSUM") as psT,
            tc.tile_pool(name="psO", bufs=2, space="PSUM") as psO,
        ):
            # ---- constants ----
            ident = persist.tile([128, 128], bf16)
            make_identity(nc, ident)
            eps_t = persist.tile([128, 1], fp32)
            nc.vector.memset(eps_t, EPS)
            # triangular mask for the diagonal 128x128 block: keep sk <= sq
            trimask = persist.tile([128, 128], bf16)
            nc.gpsimd.memset(trimask, 1.0)
            nc.gpsimd.affine_select(
                out=trimask, in_=trimask,
                compare_op=mybir.AluOpType.is_ge,
                fill=0.0, base=0,
                pattern=[[1, 128]], channel_multiplier=-1,
            )

            # ---- weights: first hst tiles, then per-chunk weight DMAs so
            # chunk-0 compute starts a few us in; wo much later ----
            hst_tiles = {}

            def prefetch_hst(t):
                if t >= NT:
                    return
                ht = hstp.tile([128, NC, 128], bf16, name=f"hst{t}", tag="hst")
                nc.sync.dma_start(out=ht, in_=hsT.ap()[:, t])
                hst_tiles[t] = ht

            prefetch_hst(0)

            wq_sb = persist.tile([128, NC, QF], bf16)
            wkv_sb = persist.tile([128, NC, 2 * D], bf16)
            tabs = {}

            def load_tabs():
                for name, t in tabs_dram.items():
                    tt = persist.tile([128, NTB, D], bf16, name=f"tab_{name}")
                    nc.sync.dma_start(
                        out=tt, in_=t.rearrange("(n p) d -> p n d", p=128))
                    tabs[name] = tt
                if shared_tabs:
                    tabs["cosk"] = tabs["cosq"]
                    tabs["sink"] = tabs["sinq"]

            for g in range(NC // 4):
                c0, c1 = 4 * g, 4 * g + 4
                nc.sync.dma_start(out=wq_sb[:, c0:c1, :],
                                  in_=wq.ap()[:, c0:c1, :])
                nc.sync.dma_start(out=wkv_sb[:, c0:c1, :],
                                  in_=wkv.ap()[:, c0:c1, :])
                if g == 1:
                    prefetch_hst(1)
                if g == 2:
                    load_tabs()

            wo_sb = persist.tile([128, HL, HID], bf16)

            # ---- persistent activations ----
            QT = [persist.tile([128, T], bf16, name=f"QT{h}") for h in range(HL)]
            KT = persist.tile([128, T], bf16)                       # [d, t]
            VA = persist.tile([128, NT, D + 1], bf16)               # [sk, d | 1]
            OT = [persist.tile([128, HL, S], bf16, name=f"OT{p}") for p in range(2)]

            # warm the PE clock (HAM) with throwaway transposes while the
            # first DMAs stream in, and pre-load the Exp ACT table set
            warm = stats.tile([128, 1], fp32, tag="warm")
            nc.scalar.activation(out=warm, in_=eps_t,
                                 func=mybir.ActivationFunctionType.Exp)
            for _ in range(40):
                wtr = psT.tile([128, 128], bf16, tag="tr")
                nc.tensor.transpose(wtr, ident, ident)

            # ================= phase 1: projections + norm + rope ============
            pending_pe = []  # deferred [transpose+copy] closures, one tile late
            # queue of PE work thunks: o_proj half-units (2 matmuls each) and
            # leftover phase-1 transpose flushes, popped between k-tiles
            oq = []
            ostate = {}  # (b, it, n) -> po psum tile with pending first half

            def rope_transpose(psum_slice, rstd, cos_t, sin_t, dstT, tcol):
                rot = work.tile([128, 128], bf16, tag="rot", bufs=6)
                shifted = work.tile([128, 128], bf16, tag="shifted", bufs=3)
                # rot = (x * rstd) * cos ; shifted = (swap(x) * rstd) * sin_f
                nc.vector.scalar_tensor_tensor(
                    out=rot, in0=psum_slice, scalar=rstd, in1=cos_t,
                    op0=MUL, op1=MUL)
                nc.vector.scalar_tensor_tensor(
                    out=shifted[:, 0:64], in0=psum_slice[:, 64:128], scalar=rstd,
                    in1=sin_t[:, 0:64], op0=MUL, op1=MUL)
                nc.vector.scalar_tensor_tensor(
                    out=shifted[:, 64:128], in0=psum_slice[:, 0:64], scalar=rstd,
                    in1=sin_t[:, 64:128], op0=MUL, op1=MUL)
                nc.gpsimd.tensor_add(out=rot, in0=rot, in1=shifted)

                def flush(rot=rot, dstT=dstT, tcol=tcol):
                    ptr = psT.tile([128, 128], bf16, tag="tr")
                    nc.tensor.transpose(ptr, rot, ident)
                    nc.scalar.copy(out=dstT[:, tcol:tcol + 128], in_=ptr)
                pending_pe.append(flush)

            for i in range(NT):
                si = i % NTB
                hst_i = hst_tiles.pop(i)
                pq = psA.tile([128, QF], fp32, tag="A")
                pkv = psB.tile([128, 2 * D], fp32, tag="B")
                for c in range(NC):
                    # back-to-back matmuls share the stationary hst chunk
                    nc.tensor.matmul(pq, hst_i[:, c, :], wq_sb[:, c, :],
                                     start=(c == 0), stop=(c == NC - 1))
                    nc.tensor.matmul(pkv, hst_i[:, c, :], wkv_sb[:, c, :],
                                     start=(c == 0), stop=(c == NC - 1))
                    if c == 24:
                        # previous tile's transposes: their DVE/GpSimd chain
                        # has had half a tile of slack, so the PE won't stall
                        for f in pending_pe:
                            f()
                        pending_pe.clear()
                prefetch_hst(i + 2)
                if i == 8:
                    # wo is needed only in phase 2; stream it during phase 1
                    nc.sync.dma_start(
                        out=wo_sb,
                        in_=wo.rearrange("(h p) f -> p h f", p=128))

                # batched RMSNorm stats for the 4 q heads + k
                ssq = stats.tile([128, 5], fp32, tag="ssq")
                scratch = work.tile([128, 128], bf16, tag="sq", bufs=1)
                for m in range(5):
                    psl = pq[:, m * D:(m + 1) * D] if m < HL else pkv[:, 0:D]
                    nc.scalar.activation(
                        out=scratch, in_=psl,
                        func=mybir.ActivationFunctionType.Square,
                        accum_out=ssq[:, m:m + 1],
                    )
                rstd = stats.tile([128, 5], fp32, tag="rstd")
                nc.scalar.activation(
                    out=rstd, in_=ssq, func=mybir.ActivationFunctionType.Sqrt,
                    bias=eps_t, scale=1.0 / D,
                )
                nc.vector.reciprocal(out=rstd, in_=rstd)

                for h in range(HL):
                    rope_transpose(
                        pq[:, h * D:(h + 1) * D], rstd[:, h:h + 1],
                        tabs["cosq"][:, si, :], tabs["sinq"][:, si, :],
                        QT[h], i * 128)
                rope_transpose(
                    pkv[:, 0:D], rstd[:, 4:5],
                    tabs["cosk"][:, si, :], tabs["sink"][:, si, :],
                    KT, i * 128)
                nc.vector.tensor_copy(out=VA[:, i, 0:D], in_=pkv[:, D:2 * D])
                nc.vector.memset(VA[:, i, D:D + 1], 1.0)

            # the last tile's transposes ride the phase-2 work queue so they
            # don't block already-ready b0 scores in the in-order PE stream
            oq.extend(pending_pe)
            pending_pe.clear()

            # ============ phase 2: attention with interleaved o_proj =========
            def emit_oproj_unit():
                if not oq:
                    return
                oq.pop(0)()

            def oproj_half(ub, it, n, half):
                if half == 0:
                    po = psO.tile([128, 512], fp32, tag="O")
                    ostate[(ub, it, n)] = po
                else:
                    po = ostate.pop((ub, it, n))
                for h in (0, 1) if half == 0 else (2, 3):
                    nc.tensor.matmul(
                        po,
                        OT[ub % 2][:, h, it * 128:(it + 1) * 128],
                        wo_sb[:, h, n * 512:(n + 1) * 512],
                        start=(h == 0), stop=(h == HL - 1))
                if half == 1:
                    ost = ostage.tile([128, 512], bf16, tag="ost")
                    nc.vector.tensor_copy(out=ost, in_=po)
                    nc.sync.dma_start(
                        out=out[ub * S + it * 128: ub * S + (it + 1) * 128,
                                n * 512:(n + 1) * 512],
                        in_=ost)

            for b in range(B):
                t0 = b * S
                k0 = b * NTB
                for j in range(4):
                    qcol = t0 + j * 512
                    # pace o_proj pops evenly across this stripe's k-iters so
                    # the queue doesn't run dry before the stripe's ACT-bound
                    # tail
                    stripe_q = len(oq)
                    stripe_iters = HL * (8 * j + 6)
                    stripe_state = [0, 0]  # [iter_idx, popped]

                    def paced_pop():
                        stripe_state[0] += 1
                        want = (stripe_q * stripe_state[0]) // stripe_iters
                        while stripe_state[1] < want and oq:
                            emit_oproj_unit()
                            stripe_state[1] += 1

                    for h in range(HL):
                        prs = {}

                        def score_exp(k):
                            s0 = k - 4 * j  # >= 0 on diagonal tiles
                            off = 128 * s0 if s0 > 0 else 0
                            ps_s = psA.tile([128, 512], fp32, tag="A")
                            nc.tensor.matmul(
                                ps_s[:, off:512],
                                KT[:, t0 + k * 128: t0 + (k + 1) * 128],
                                QT[h][:, qcol + off: qcol + 512],
                                start=True, stop=True)
                            pr = prp.tile([128, 512], bf16, tag="pr")
                            nc.scalar.activation(
                                out=pr[:, off:512], in_=ps_s[:, off:512],
                                func=mybir.ActivationFunctionType.Exp,
                                scale=SCALE)
                            if s0 >= 0:
                                nc.vector.tensor_mul(
                                    out=pr[:, off:off + 128],
                                    in0=pr[:, off:off + 128], in1=trimask)
                            prs[k] = pr

                        def epilogue(s, opv):
                            recip = stats.tile([128, 1], fp32, tag="recip")
                            nc.vector.reciprocal(out=recip, in_=opv[:, D:D + 1])
                            onorm = work.tile([128, 128], bf16, tag="onorm")
                            nc.vector.tensor_scalar_mul(
                                out=onorm, in0=opv[:, 0:D], scalar1=recip)
                            ptr = psT.tile([128, 128], bf16, tag="tr")
                            nc.tensor.transpose(ptr, onorm, ident)
                            # DVE, not ACT: keeps the next stripe's exps at
                            # the head of ACT's in-order queue
                            nc.vector.tensor_copy(
                                out=OT[b % 2][:, h, j * 512 + s * 128:
                                              j * 512 + (s + 1) * 128],
                                in_=ptr)

                        # pass A: s = 0,1
                        opv01 = [psB.tile([128, D + 1], fp32, tag="B",
                                          name=f"opv{s}") for s in (0, 1)]
                        for k in range(4 * j + 2):
                            score_exp(k)
                            for s in (0, 1):
                                if k <= 4 * j + s:
                                    nc.tensor.matmul(
                                        opv01[s],
                                        prs[k][:, s * 128:(s + 1) * 128],
                                        VA[:, k0 + k, :],
                                        start=(k == 0), stop=(k == 4 * j + s))
                            paced_pop()
                        epilogue(0, opv01[0])
                        epilogue(1, opv01[1])

                        # pass B: s = 2,3 (reuses cached pr tiles)
                        opv23 = [psB.tile([128, D + 1], fp32, tag="B",
                                          name=f"opv{s}") for s in (2, 3)]
                        for k in range(4 * j + 4):
                            if k >= 4 * j + 2:
                                score_exp(k)
                            for s in (2, 3):
                                if k <= 4 * j + s:
                                    nc.tensor.matmul(
                                        opv23[s - 2],
                                        prs[k][:, s * 128:(s + 1) * 128],
                                        VA[:, k0 + k, :],
                                        start=(k == 0), stop=(k == 4 * j + s))
                            paced_pop()
                        epilogue(2, opv23[0])
                        epilogue(3, opv23[1])

                    # enqueue this stripe's o_proj half-units (it is batch-local)
                    for it in range(4 * j, 4 * j + 4):
                        for n in range(HID // 512):
                            for half in (0, 1):
                                oq.append(
                                    lambda ub=b, it=it, n=n, half=half:
                                    oproj_half(ub, it, n, half))

            while oq:
                emit_oproj_unit()

    nc.finalize()
    return nc


def _get_nc(shared_tabs):
    key = ("nc", shared_tabs)
    if key not in _NC_CACHE:
        _NC_CACHE[key] = _build(shared_tabs)
    return _NC_CACHE[key]


def _host_prep(hidden_states, wq, wk, wv, wo, q_norm_w, k_norm_w, position_ids,
               shared_tabs):
    bf = ml_dtypes.bfloat16
    hs = np.asarray(hidden_states, dtype=np.float32).reshape(T, HID)
    # pre-tile for the kernel: hsT[p, i, c, t] = hs[128*i + t, 128*c + p]
    hsT = np.ascontiguousarray(
        hs.reshape(NT, 128, NC, 128).transpose(3, 0, 2, 1)).astype(bf)

    # RoPE tables with norm weights folded in (positions are identical
    # across batches for this problem's arange position_ids).
    pos = np.asarray(position_ids)[0].astype(np.float64)
    inv_freq = 1.0 / (THETA ** (np.arange(0, D, 2, dtype=np.float64) / D))
    ang = pos[:, None] * inv_freq
    emb = np.concatenate([ang, ang], axis=-1)
    cos = np.cos(emb).astype(np.float32)
    sin = np.sin(emb).astype(np.float32)

    def fold(w):
        w = np.asarray(w, dtype=np.float32)
        w_shift = np.concatenate([w[D // 2:], w[:D // 2]])
        sgn = np.concatenate([-np.ones(D // 2, np.float32),
                              np.ones(D // 2, np.float32)])
        return (cos * w).astype(bf), (sin * w_shift * sgn).astype(bf)

    cq, sq_ = fold(q_norm_w)

    wq = np.asarray(wq, dtype=np.float32)
    wk = np.asarray(wk, dtype=np.float32)
    wv = np.asarray(wv, dtype=np.float32)
    wo = np.asarray(wo, dtype=np.float32)

    in_maps = []
    for c in range(NCORES):
        qs = slice(c * QF, (c + 1) * QF)
        ks = slice(c * D, (c + 1) * D)
        wq_c = wq[:, qs].reshape(NC, 128, QF).transpose(1, 0, 2)
        wkv_c = np.concatenate([wk[:, ks], wv[:, ks]],
                               axis=1).reshape(NC, 128, 2 * D).transpose(1, 0, 2)
        m = {
            "hsT": hsT,
            "wq": np.ascontiguousarray(wq_c).astype(bf),
            "wkv": np.ascontiguousarray(wkv_c).astype(bf),
            "wo": np.ascontiguousarray(wo[qs, :]).astype(bf),
            "cosq": cq, "sinq": sq_,
        }
        if not shared_tabs:
            ck, sk_ = fold(k_norm_w)
            m["cosk"] = ck
            m["sink"] = sk_
        in_maps.append(m)
    return in_maps


def kernel(hidden_states, wq, wk, wv, wo, q_norm_w, k_norm_w, position_ids,
           _trace=False):
    from concourse.bass_utils import run_bass_kernel_spmd

    shared_tabs = bool(np.array_equal(np.asarray(q_norm_w),
                                      np.asarray(k_norm_w)))
    nc = _get_nc(shared_tabs)
    in_maps = _host_prep(hidden_states, wq, wk, wv, wo,
                         q_norm_w, k_norm_w, position_ids, shared_tabs)
    res = run_bass_kernel_spmd(nc, in_maps, core_ids=list(range(NCORES)),
                               trace=_trace)
    total = np.zeros((T, HID), dtype=np.float32)
    for r in res.results:
        total += r["out"].astype(np.float32)
    out = total.reshape(B, S, HID)
    if _trace:
        return out, res
    return out


# revision 55
# speedup vs baseline: 1.0124x; 1.0124x over previous
"""GQA attention layer (dense_transformer) on 8 Trainium2 NeuronCores.

Tensor-parallel over heads: each core gets 4 q-heads + 1 kv-head (shard of
wq/wk/wv output dims and wo input dim), hidden_states replicated; partial
o_proj outputs are summed on the host (the all-reduce).

Structure (all matmuls bf16, fp32 PSUM; measured ~774us vs 1007us baseline):
  phase 1: per 128-token tile, qkv projections from host-PRETILED hsT
    and wq/wkv (contiguous multi-KB DMA lines per partition); 4-chunk
    weight-group DMAs + PE-warmup transposes so compute starts hot; fused
    RMSNorm+RoPE via scalar_tensor_tensor (rope add on GpSimd); the PE
    transposes of tile i are deferred into tile i+1's matmul stream so the
    in-order PE queue never waits on the DVE/GpSimd rope chain.
  phase 2: per batch, j-stripe (512 q cols) outer, head inner.
    scoresT = k @ qT per k-tile with causal width-trimming on diagonal
    tiles; exp on ACT (table preloaded in phase 1); triangular mask on DVE;
    PV split in two passes (s=0,1 then s=2,3) so only 2 PSUM banks hold PV
    accumulators; probs tiles persist in SBUF for pass B reuse.  o_proj
    2-matmul half-units are interleaved into the attention stream via a
    paced work queue so the PE stays busy during exp stalls.  o_proj
    results copied to bf16 on DVE (not ACT) and DMA'd out bf16; host sums
    partials in fp32 (the tensor-parallel all-reduce).
"""

import numpy as np
import ml_dtypes

H, KV, D, HID = 32, 8, 128, 4096
B, S = 2, 2048
T = B * S
NCORES = 8
HL = H // NCORES          # 4 q heads per core
QF = HL * D               # 512
EPS = 1e-6
THETA = 10000.0
SCALE = 1.0 / float(np.sqrt(D))

NT = T // 128             # 32 token tiles
NTB = S // 128            # 16 token tiles per batch
NC = HID // 128           # 32 contraction chunks

_NC_CACHE = {}


def _build(shared_tabs):
    import concourse.bacc as bacc
    import concourse.mybir as mybir
    import concourse.tile as tile
    from concourse.masks import make_identity

    fp32 = mybir.dt.float32
    bf16 = mybir.dt.bfloat16
    MUL = mybir.AluOpType.mult

    nc = bacc.Bacc("TRN2", target_bir_lowering=False)

    # host-pretiled: hsT[p, i, c, t] = hs[128*i + t, 128*c + p], so a token
    # tile's DMA is one contiguous 8KB run per partition
    hsT = nc.dram_tensor("hsT", [128, NT, NC, 128], bf16, kind="ExternalInput")
    # host-pretiled like hsT: wq[p, c, f] = wq_full[128*c + p, f]
    wq = nc.dram_tensor("wq", [128, NC, QF], bf16, kind="ExternalInput")
    wkv = nc.dram_tensor("wkv", [128, NC, 2 * D], bf16, kind="ExternalInput")
    wo = nc.dram_tensor("wo", [QF, HID], bf16, kind="ExternalInput")
    tab_names = ["cosq", "sinq"] if shared_tabs else ["cosq", "sinq", "cosk", "sink"]
    tabs_dram = {n: nc.dram_tensor(n, [S, D], bf16, kind="ExternalInput")
                 for n in tab_names}
    out = nc.dram_tensor("out", [T, HID], bf16, kind="ExternalOutput")

    # the non-shared-tabs variant needs 8KB SBUF for two extra RoPE tables;
    # pay for it with a shallower hst prefetch ring (slower, rarely used)
    hst_bufs = 2 if shared_tabs else 1
    with tile.TileContext(nc) as tc:
        with (
            tc.tile_pool(name="persist", bufs=1) as persist,
            tc.tile_pool(name="hst", bufs=hst_bufs) as hstp,
            tc.tile_pool(name="work", bufs=3) as work,
            tc.tile_pool(name="prp", bufs=16) as prp,
            tc.tile_pool(name="stats", bufs=4) as stats,
            tc.tile_pool(name="ostage", bufs=3) as ostage,
            tc.tile_pool(name="psA", bufs=2, space="PSUM") as psA,
            tc.tile_pool(name="psB", bufs=2, space="PSUM") as psB,
            tc.tile_pool(name="psT", bufs=2, space="PSUM") as psT,
            tc.tile_pool(name="psO", bufs=2, space="PSUM") as psO,
        ):
            # ---- constants ----
            ident = persist.tile([128, 128], bf16)
            make_identity(nc, ident)
            eps_t = persist.tile([128, 1], fp32)
            nc.vector.memset(eps_t, EPS)
            # triangular mask for the diagonal 128x128 block: keep sk <= sq
            trimask = persist.tile([128, 128], bf16)
            nc.gpsimd.memset(trimask, 1.0)
            nc.gpsimd.affine_select(
                out=trimask, in_=trimask,
                compare_op=mybir.AluOpType.is_ge,
                fill=0.0, base=0,
                pattern=[[1, 128]], channel_multiplier=-1,
            )

            # ---- weights: first hst tiles, then per-chunk weight DMAs so
            # chunk-0 compute starts a few us in; wo much later ----
            hst_tiles = {}

            def prefetch_hst(t):
                if t >= NT:
                    return
                ht = hstp.tile([128, NC, 128], bf16, name=f"hst{t}", tag="hst")
                nc.sync.dma_start(out=ht, in_=hsT.ap()[:, t])
                hst_tiles[t] = ht

            prefetch_hst(0)

            wq_sb = persist.tile([128, NC, QF], bf16)
            wkv_sb = persist.tile([128, NC, 2 * D], bf16)
            tabs = {}

            def load_tabs():
                for name, t in tabs_dram.items():
                    tt = persist.tile([128, NTB, D], bf16, name=f"tab_{name}")
                    nc.sync.dma_start(
                        out=tt, in_=t.rearrange("(n p) d -> p n d", p=128))
                    tabs[name] = tt
                if shared_tabs:
                    tabs["cosk"] = tabs["cosq"]
                    tabs["sink"] = tabs["sinq"]

            for g in range(NC // 4):
                c0, c1 = 4 * g, 4 * g + 4
                nc.sync.dma_start(out=wq_sb[:, c0:c1, :],
                                  in_=wq.ap()[:, c0:c1, :])
                nc.sync.dma_start(out=wkv_sb[:, c0:c1, :],
                                  in_=wkv.ap()[:, c0:c1, :])
                if g == 1:
                    prefetch_hst(1)
                if g == 2:
                    load_tabs()

            wo_sb = persist.tile([128, HL, HID], bf16)

            # ---- persistent activations ----
            QT = [persist.tile([128, T], bf16, name=f"QT{h}") for h in range(HL)]
            KT = persist.tile([128, T], bf16)                       # [d, t]
            VA = persist.tile([128, NT, D + 1], bf16)               # [sk, d | 1]
            OT = [persist.tile([128, HL, S], bf16, name=f"OT{p}") for p in range(2)]

            # warm the PE clock (HAM) with throwaway transposes while the
            # first DMAs stream in, and pre-load the Exp ACT table set
            warm = stats.tile([128, 1], fp32, tag="warm")
            nc.scalar.activation(out=warm, in_=eps_t,
                                 func=mybir.ActivationFunctionType.Exp)
            for _ in range(40):
                wtr = psT.tile([128, 128], bf16, tag="tr")
                nc.tensor.transpose(wtr, ident, ident)

            # ================= phase 1: projections + norm + rope ============
            pending_pe = []  # deferred [transpose+copy] closures, one tile late
            # queue of PE work thunks: o_proj half-units (2 matmuls each) and
            # leftover phase-1 transpose flushes, popped between k-tiles
            oq = []
            ostate = {}  # (b, it, n) -> po psum tile with pending first half

            def rope_transpose(psum_slice, rstd, cos_t, sin_t, dstT, tcol):
                rot = work.tile([128, 128], bf16, tag="rot", bufs=6)
                shifted = work.tile([128, 128], bf16, tag="shifted", bufs=3)
                # rot = (x * rstd) * cos ; shifted = (swap(x) * rstd) * sin_f
                nc.vector.scalar_tensor_tensor(
                    out=rot, in0=psum_slice, scalar=rstd, in1=cos_t,
                    op0=MUL, op1=MUL)
                nc.vector.scalar_tensor_tensor(
                    out=shifted[:, 0:64], in0=psum_slice[:, 64:128], scalar=rstd,
                    in1=sin_t[:, 0:64], op0=MUL, op1=MUL)
                nc.vector.scalar_tensor_tensor(
                    out=shifted[:, 64:128], in0=psum_slice[:, 0:64], scalar=rstd,
                    in1=sin_t[:, 64:128], op0=MUL, op1=MUL)
                nc.gpsimd.tensor_add(out=rot, in0=rot, in1=shifted)

                def flush(rot=rot, dstT=dstT, tcol=tcol):
                    ptr = psT.tile([128, 128], bf16, tag="tr")
                    nc.tensor.transpose(ptr, rot, ident)
                    nc.scalar.copy(out=dstT[:, tcol:tcol + 128], in_=ptr)
                pending_pe.append(flush)

            for i in range(NT):
                si = i % NTB
                hst_i = hst_tiles.pop(i)
                pq = psA.tile([128, QF], fp32, tag="A")
                pkv = psB.tile([128, 2 * D], fp32, tag="B")
                for c in range(NC):
                    # back-to-back matmuls share the stationary hst chunk
                    nc.tensor.matmul(pq, hst_i[:, c, :], wq_sb[:, c, :],
                                     start=(c == 0), stop=(c == NC - 1))
                    nc.tensor.matmul(pkv, hst_i[:, c, :], wkv_sb[:, c, :],
                                     start=(c == 0), stop=(c == NC - 1))
                    if c == 24:
                        # previous tile's transposes: their DVE/GpSimd chain
                        # has had half a tile of slack, so the PE won't stall
                        for f in pending_pe:
                            f()
                        pending_pe.clear()
                prefetch_hst(i + 2)
                if i == 8:
                    # wo is needed only in phase 2; stream it during phase 1
                    nc.sync.dma_start(
                        out=wo_sb,
                        in_=wo.rearrange("(h p) f -> p h f", p=128))

                # batched RMSNorm stats for the 4 q heads + k
                ssq = stats.tile([128, 5], fp32, tag="ssq")
                scratch = work.tile([128, 128], bf16, tag="sq", bufs=1)
                for m in range(5):
                    psl = pq[:, m * D:(m + 1) * D] if m < HL else pkv[:, 0:D]
                    nc.scalar.activation(
                        out=scratch, in_=psl,
                        func=mybir.ActivationFunctionType.Square,
                        accum_out=ssq[:, m:m + 1],
                    )
                rstd = stats.tile([128, 5], fp32, tag="rstd")
                nc.scalar.activation(
                    out=rstd, in_=ssq, func=mybir.ActivationFunctionType.Sqrt,
                    bias=eps_t, scale=1.0 / D,
                )
                nc.vector.reciprocal(out=rstd, in_=rstd)

                for h in range(HL):
                    rope_transpose(
                        pq[:, h * D:(h + 1) * D], rstd[:, h:h + 1],
                        tabs["cosq"][:, si, :], tabs["sinq"][:, si, :],
                        QT[h], i * 128)
                rope_transpose(
                    pkv[:, 0:D], rstd[:, 4:5],
                    tabs["cosk"][:, si, :], tabs["sink"][:, si, :],
                    KT, i * 128)
                nc.vector.tensor_copy(out=VA[:, i, 0:D], in_=pkv[:, D:2 * D])
                nc.vector.memset(VA[:, i, D:D + 1], 1.0)

            # the last tile's transposes ride the phase-2 work queue so they
            # don't block already-ready b0 scores in the in-order PE stream
            oq.extend(pending_pe)
            pending_pe.clear()

            # ============ phase 2: attention with interleaved o_proj =========
            def emit_oproj_unit():
                if not oq:
                    return
                oq.pop(0)()

            def oproj_half(ub, it, n, half):
                if half == 0:
                    po = psO.tile([128, 512], fp32, tag="O")
                    ostate[(ub, it, n)] = po
                else:
                    po = ostate.pop((ub, it, n))
                for h in (0, 1) if half == 0 else (2, 3):
                    nc.tensor.matmul(
                        po,
                        OT[ub % 2][:, h, it * 128:(it + 1) * 128],
                        wo_sb[:, h, n * 512:(n + 1) * 512],
                        start=(h == 0), stop=(h == HL - 1))
                if half == 1:
                    ost = ostage.tile([128, 512], bf16, tag="ost")
                    nc.vector.tensor_copy(out=ost, in_=po)
                    nc.sync.dma_start(
                        out=out[ub * S + it * 128: ub * S + (it + 1) * 128,
                                n * 512:(n + 1) * 512],
                        in_=ost)

            for b in range(B):
                t0 = b * S
                k0 = b * NTB
                for j in range(4):
                    qcol = t0 + j * 512
                    # pace o_proj pops evenly across this stripe's k-iters so
                    # the queue doesn't run dry before the stripe's ACT-bound
                    # tail
                    stripe_q = len(oq)
                    stripe_iters = HL * (8 * j + 6)
                    stripe_state = [0, 0]  # [iter_idx, popped]

                    def paced_pop():
                        stripe_state[0] += 1
                        want = (stripe_q * stripe_state[0]) // stripe_iters
                        while stripe_state[1] < want and oq:
                            emit_oproj_unit()
                            stripe_state[1] += 1

                    for h in range(HL):
                        prs = {}

                        def score_exp(k):
                            s0 = k - 4 * j  # >= 0 on diagonal tiles
                            off = 128 * s0 if s0 > 0 else 0
                            ps_s = psA.tile([128, 512], fp32, tag="A")
                            nc.tensor.matmul(
                                ps_s[:, off:512],
                                KT[:, t0 + k * 128: t0 + (k + 1) * 128],
                                QT[h][:, qcol + off: qcol + 512],
                                start=True, stop=True)
                            pr = prp.tile([128, 512], bf16, tag="pr")
                            nc.scalar.activation(
                                out=pr[:, off:512], in_=ps_s[:, off:512],
                                func=mybir.ActivationFunctionType.Exp,
                                scale=SCALE)
                            if s0 >= 0:
                                nc.vector.tensor_mul(
                                    out=pr[:, off:off + 128],
                                    in0=pr[:, off:off + 128], in1=trimask)
                            prs[k] = pr

                        def epilogue(s, opv):
                            recip = stats.tile([128, 1], fp32, tag="recip")
                            nc.vector.reciprocal(out=recip, in_=opv[:, D:D + 1])
                            onorm = work.tile([128, 128], bf16, tag="onorm")
                            nc.vector.tensor_scalar_mul(
                                out=onorm, in0=opv[:, 0:D], scalar1=recip)
                            ptr = psT.tile([128, 128], bf16, tag="tr")
                            nc.tensor.transpose(ptr, onorm, ident)
                            # DVE, not ACT: keeps the next stripe's exps at
                            # the head of ACT's in-order queue
                            nc.vector.tensor_copy(
                                out=OT[b % 2][:, h, j * 512 + s * 128:
                                              j * 512 + (s + 1) * 128],
                                in_=ptr)

                        # pass A: s = 0,1
                        opv01 = [psB.tile([128, D + 1], fp32, tag="B",
                                          name=f"opv{s}") for s in (0, 1)]
                        for k in range(4 * j + 2):
                            score_exp(k)
                            for s in (0, 1):
                                if k <= 4 * j + s:
                                    nc.tensor.matmul(
                                        opv01[s],
                                        prs[k][:, s * 128:(s + 1) * 128],
                                        VA[:, k0 + k, :],
                                        start=(k == 0), stop=(k == 4 * j + s))
                            paced_pop()
                        epilogue(0, opv01[0])
                        epilogue(1, opv01[1])

                        # pass B: s = 2,3 (reuses cached pr tiles)
                        opv23 = [psB.tile([128, D + 1], fp32, tag="B",
                                          name=f"opv{s}") for s in (2, 3)]
                        for k in range(4 * j + 4):
                            if k >= 4 * j + 2:
                                score_exp(k)
                            for s in (2, 3):
                                if k <= 4 * j + s:
                                    nc.tensor.matmul(
                                        opv23[s - 2],
                                        prs[k][:, s * 128:(s + 1) * 128],
                                        VA[:, k0 + k, :],
                                        start=(k == 0), stop=(k == 4 * j + s))
                            paced_pop()
                        epilogue(2, opv23[0])
                        epilogue(3, opv23[1])

                    # enqueue this stripe's o_proj half-units (it is batch-local)
                    for it in range(4 * j, 4 * j + 4):
                        for n in range(HID // 512):
                            for half in (0, 1):
                                oq.append(
                                    lambda ub=b, it=it, n=n, half=half:
                                    oproj_half(ub, it, n, half))

            while oq:
                emit_oproj_unit()

    nc.finalize()
    return nc


def _get_nc(shared_tabs):
    key = ("nc", shared_tabs)
    if key not in _NC_CACHE:
        _NC_CACHE[key] = _build(shared_tabs)
    return _NC_CACHE[key]


def _host_prep(hidden_states, wq, wk, wv, wo, q_norm_w, k_norm_w, position_ids,
               shared_tabs):
    bf = ml_dtypes.bfloat16
    hs = np.asarray(hidden_states, dtype=np.float32).reshape(T, HID)
    # pre-tile for the kernel: hsT[p, i, c, t] = hs[128*i + t, 128*c + p]
    hsT = np.ascontiguousarray(
        hs.reshape(NT, 128, NC, 128).transpose(3, 0, 2, 1)).astype(bf)

    # RoPE tables with norm weights folded in (positions are identical
    # across batches for this problem's arange position_ids).
    pos = np.asarray(position_ids)[0].astype(np.float64)
    inv_freq = 1.0 / (THETA ** (np.arange(0, D, 2, dtype=np.float64) / D))
    ang = pos[:, None] * inv_freq
    emb = np.concatenate([ang, ang], axis=-1)
    cos = np.cos(emb).astype(np.float32)
    sin = np.sin(emb).astype(np.float32)

    def fold(w):
        w = np.asarray(w, dtype=np.float32)
        w_shift = np.concatenate([w[D // 2:], w[:D // 2]])
        sgn = np.concatenate([-np.ones(D // 2, np.float32),
                              np.ones(D // 2, np.float32)])
        return (cos * w).astype(bf), (sin * w_shift * sgn).astype(bf)

    cq, sq_ = fold(q_norm_w)

    wq = np.asarray(wq, dtype=np.float32)
    wk = np.asarray(wk, dtype=np.float32)
    wv = np.asarray(wv, dtype=np.float32)
    wo = np.asarray(wo, dtype=np.float32)

    in_maps = []
    for c in range(NCORES):
        qs = slice(c * QF, (c + 1) * QF)
        ks = slice(c * D, (c + 1) * D)
        wq_c = wq[:, qs].reshape(NC, 128, QF).transpose(1, 0, 2)
        wkv_c = np.concatenate([wk[:, ks], wv[:, ks]],
                               axis=1).reshape(NC, 128, 2 * D).transpose(1, 0, 2)
        m = {
            "hsT": hsT,
            "wq": np.ascontiguousarray(wq_c).astype(bf),
            "wkv": np.ascontiguousarray(wkv_c).astype(bf),
            "wo": np.ascontiguousarray(wo[qs, :]).astype(bf),
            "cosq": cq, "sinq": sq_,
        }
        if not shared_tabs:
            ck, sk_ = fold(k_norm_w)
            m["cosk"] = ck
            m["sink"] = sk_
        in_maps.append(m)
    return in_maps


def kernel(hidden_states, wq, wk, wv, wo, q_norm_w, k_norm_w, position_ids,
           _trace=False):
    from concourse.bass_utils import run_bass_kernel_spmd

    shared_tabs = bool(np.array_equal(np.asarray(q_norm_w),
                                      np.asarray(k_norm_w)))
    nc = _get_nc(shared_tabs)
    in_maps = _host_prep(hidden_states, wq, wk, wv, wo,
                         q_norm_w, k_norm_w, position_ids, shared_tabs)
    res = run_bass_kernel_spmd(nc, in_maps, core_ids=list(range(NCORES)),
                               trace=_trace)
    total = np.zeros((T, HID), dtype=np.float32)
    for r in res.results:
        total += r["out"].astype(np.float32)
    out = total.reshape(B, S, HID)
    if _trace:
        return out, res
    return out
